# revision 12
# baseline (speedup 1.0000x reference)
"""Trainium2 Bass kernel for nn_NegUniform (topk_masking).

Computes: L2-normalize feature & negative_features, sims = f_hat @ negs_hat^T
per negative set j (masked same-class for j==idx), top-16 per row, softmax
entropy over the J axis, decay-weighted mean + log(J).

Sharding: data-parallel over the n (row) dimension of `feature` across 8
NeuronCores; negative_features / target replicated. Each core returns
per-row-group partial sums [128, 4]; the host reduces them to the scalar.

Per-core pipeline:
  - negs (host-cast fp16) are DMA-xbar-transposed DRAM->SBUF to [d, m] layout
  - column norms via ACT square + PE ones-matmul, rsqrt on small tiles,
    partition-broadcast, one fp16 multiply -> normalized negsT
  - feature slice normalized in f32, cast fp16, xbar-transposed -> fT
  - sims tile [128 rows, 1024 cands] = fp16 matmul into PSUM f32; same-class
    mask folded in as a rank-4 one-hot matmul accumulated into PSUM (j==idx)
  - top-16 per row: DVE max8 per 1024-chunk directly from PSUM (union of
    top-8s), then max8 + match_replace + max8 over the 32 candidates
  - softmax-entropy over j in f32 on [128, 16] tiles, decay-weighted row sums
"""

import math
import sys

import numpy as np

for _p in ("/opt/trn_rl_repo",):
    if _p not in sys.path:
        sys.path.insert(0, _p)

N = 4096
D = 128
J = 4
NCORES = 8
NLOC = N // NCORES          # 512 rows per core
RT = NLOC // 128            # 4 row-tiles per core
K = 16
TEMP = 0.01
V = 0.95
MASK_NEG = -60000.0         # fp16-representable; dominates any cosine sim
CHUNK = 1024                # max8 scan chunk (2 PSUM banks)
NCHUNK = N // CHUNK         # 4 scan chunks per row-tile

_BUILD_CACHE = {}
LAST_RESULT = None  # BassKernelResults of the most recent kernel() call


def _build(idx: int):
    if idx in _BUILD_CACHE:
        return _BUILD_CACHE[idx]

    import concourse.bacc as bacc
    import concourse.tile as tile
    import concourse.mybir as mybir
    from concourse import bass_isa

    f32 = mybir.dt.float32
    f16 = mybir.dt.float16
    AF = mybir.ActivationFunctionType
    OP = mybir.AluOpType

    nc = bacc.Bacc(
        "TRN2",
        target_bir_lowering=False,
        debug=False,
        enable_asserts=False,
        num_devices=NCORES,
    )

    feat = nc.dram_tensor("feat", [NLOC, D], f32, kind="ExternalInput").ap()
    negs16 = nc.dram_tensor("negs16", [J, N, D], f16, kind="ExternalInput").ap()
    maskL = nc.dram_tensor("maskL", [J, NLOC], f16, kind="ExternalInput").ap()
    onehotR = nc.dram_tensor("onehotR", [J, N], f16, kind="ExternalInput").ap()
    decayb = nc.dram_tensor("decayb", [128, K], f32, kind="ExternalInput").ap()
    out = nc.dram_tensor("out", [128, RT], f32, kind="ExternalOutput").ap()

    with tile.TileContext(nc) as tc:
        with (
            tc.tile_pool(name="consts", bufs=1) as cpool,
            tc.tile_pool(name="fprep", bufs=2) as fpool,
            tc.tile_pool(name="nprep", bufs=2) as npool,
            tc.tile_pool(name="negsT", bufs=2) as ntpool,
            tc.tile_pool(name="small", bufs=3) as spool,
            tc.tile_pool(name="tops", bufs=2 * J * RT) as tpool,
            tc.tile_pool(name="ent", bufs=4) as epool,
            tc.tile_pool(name="psums", bufs=4, space="PSUM") as psp,
        ):
            # ---- constants ----
            decay_t = cpool.tile([128, K], f32)
            nc.sync.dma_start(decay_t, decayb)
            maskL_t = cpool.tile([J, NLOC], f16)
            nc.sync.dma_start(maskL_t, maskL)
            onehotR_t = cpool.tile([J, N], f16)
            nc.sync.dma_start(onehotR_t, onehotR)
            partials = cpool.tile([128, RT], f32)

            # ---- feature prep: normalize f32, cast fp16, transpose ----
            fT = cpool.tile([128, NLOC], f16)  # [d, n_local]
            fhat_tiles = []
            fnrm2 = spool.tile([128, RT], f32, tag="fnrm")
            fscr = fpool.tile([128, D], f32, tag="fscr")
            ftiles = []
            for t in range(RT):
                ft = fpool.tile([128, D], f32, tag=f"ft{t}")
                nc.sync.dma_start(ft, feat[t * 128:(t + 1) * 128, :])
                ftiles.append(ft)
                nc.vector.tensor_mul(fscr, ft, ft)
                nc.vector.tensor_reduce(
                    out=fnrm2[:, t:t + 1], in_=fscr, op=OP.add,
                    axis=mybir.AxisListType.X,
                )
            fnrmS = spool.tile([128, RT], f32, tag="fnrmS")
            nc.scalar.activation(out=fnrmS, in_=fnrm2, func=AF.Sqrt)
            frs = spool.tile([128, RT], f32, tag="frs")
            nc.vector.reciprocal(frs, fnrmS)
            for t in range(RT):
                fh = fpool.tile([128, D], f16, tag=f"fh{t}")
                nc.vector.tensor_scalar(
                    out=fh, in0=ftiles[t], scalar1=frs[:, t:t + 1], scalar2=None,
                    op0=OP.mult,
                )
                fhat_tiles.append(fh)
                nc.sync.dma_start_transpose(fT[:, t * 128:(t + 1) * 128], fh)

            # ---- negs prep + main loop, per j ----
            tops = {}
            for j in range(J):
                raw = npool.tile([128, N], f16, tag="raw")  # [d, m] unnormalized
                for c in range(4):
                    nc.sync.dma_start_transpose(
                        raw[:, c * 1024:(c + 1) * 1024],
                        negs16[j, c * 1024:(c + 1) * 1024, :],
                    )
                sq = npool.tile([128, N], f16, tag="sq")
                nc.scalar.activation(out=sq, in_=raw, func=AF.Square)
                nrm2a = npool.tile([128, N], f32, tag="nrm2a")
                nc.gpsimd.partition_all_reduce(
                    nrm2a, sq, channels=128, reduce_op=bass_isa.ReduceOp.add,
                )
                nrm2 = npool.tile([128, 32], f32, tag="nrm2")
                nc.sync.dma_start(nrm2, nrm2a[0:1, :])  # [1,4096] -> [128,32]
                nrmS = npool.tile([128, 32], f32, tag="nrmS")
                nc.scalar.activation(out=nrmS, in_=nrm2, func=AF.Sqrt)
                rsf = npool.tile([128, 32], f32, tag="rsf")
                nc.vector.reciprocal(rsf, nrmS)
                rsh = npool.tile([128, 32], f16, tag="rsh")
                nc.vector.tensor_copy(rsh, rsf)
                rs1 = npool.tile([1, N], f16, tag="rs1")
                nc.sync.dma_start(rs1, rsh)  # partition-major fold: rs1[0, p*32+t]
                rsb = npool.tile([128, N], f16, tag="rsb")
                nc.gpsimd.partition_broadcast(rsb, rs1)
                negsT = ntpool.tile([128, N], f16, tag="negsT")
                nc.vector.tensor_mul(negsT, raw, rsb)

                # ---- sims + topk for each row-tile ----
                for t in range(RT):
                    cand = spool.tile([128, 8 * NCHUNK], f32, tag="cand")
                    for c in range(NCHUNK):
                        ps = psp.tile([128, CHUNK], f32, tag="sims")
                        for h in range(CHUNK // 512):
                            m0 = c * CHUNK + h * 512
                            nc.tensor.matmul(
                                ps[:, h * 512:(h + 1) * 512],
                                lhsT=fT[:, t * 128:(t + 1) * 128],
                                rhs=negsT[:, m0:m0 + 512],
                                start=True, stop=(j != idx),
                            )
                            if j == idx:
                                nc.tensor.matmul(
                                    ps[:, h * 512:(h + 1) * 512],
                                    lhsT=maskL_t[:, t * 128:(t + 1) * 128],
                                    rhs=onehotR_t[:, m0:m0 + 512],
                                    start=False, stop=True,
                                )
                        nc.vector.max(out=cand[:, c * 8:(c + 1) * 8], in_=ps)
                    top16 = tpool.tile([128, K], f32, tag=f"top_{j}_{t}")
                    rep = spool.tile([128, 8 * NCHUNK], f32, tag="rep")
                    nc.vector.max(out=top16[:, 0:8], in_=cand)
                    nc.vector.match_replace(
                        out=rep, in_to_replace=top16[:, 0:8], in_values=cand,
                        imm_value=-1e30,
                    )
                    nc.vector.max(out=top16[:, 8:16], in_=rep)
                    tops[(j, t)] = top16

            # ---- softmax-entropy over j, decay-weighted row sums ----
            for t in range(RT):
                v = [tops[(j, t)] for j in range(J)]
                t01 = epool.tile([128, K], f32, tag="t01")
                t23 = epool.tile([128, K], f32, tag="t23")
                m = epool.tile([128, K], f32, tag="m")
                nc.vector.tensor_max(t01, v[0], v[1])
                nc.vector.tensor_max(t23, v[2], v[3])
                nc.vector.tensor_max(m, t01, t23)
                d_ = [epool.tile([128, K], f32, tag=f"d{j}", name=f"d{j}_{t}")
                      for j in range(J)]
                e_ = [epool.tile([128, K], f32, tag=f"e{j}", name=f"e{j}_{t}")
                      for j in range(J)]
                for j in range(J):
                    nc.vector.tensor_sub(d_[j], v[j], m)
                    nc.scalar.activation(out=e_[j], in_=d_[j], func=AF.Exp,
                                         scale=1.0 / TEMP)
                s01 = epool.tile([128, K], f32, tag="s01")
                s23 = epool.tile([128, K], f32, tag="s23")
                S = epool.tile([128, K], f32, tag="S")
                nc.vector.tensor_add(s01, e_[0], e_[1])
                nc.vector.tensor_add(s23, e_[2], e_[3])
                nc.vector.tensor_add(S, s01, s23)
                lnS = epool.tile([128, K], f32, tag="lnS")
                nc.scalar.activation(out=lnS, in_=S, func=AF.Ln)
                R = epool.tile([128, K], f32, tag="R")
                nc.vector.reciprocal(R, S)
                # w = sum_j e_j * d_j  (reuse d_ as scratch)
                for j in range(J):
                    nc.vector.tensor_mul(d_[j], e_[j], d_[j])
                nc.vector.tensor_add(d_[0], d_[0], d_[1])
                nc.vector.tensor_add(d_[2], d_[2], d_[3])
                nc.vector.tensor_add(d_[0], d_[0], d_[2])
                # ent = (w * R) / TEMP - lnS
                nc.vector.tensor_mul(d_[0], d_[0], R)
                nc.vector.tensor_scalar(
                    out=d_[0], in0=d_[0], scalar1=1.0 / TEMP, scalar2=None,
                    op0=OP.mult,
                )
                nc.vector.tensor_sub(d_[0], d_[0], lnS)
                escr = epool.tile([128, K], f32, tag="escr")
                nc.vector.tensor_mul(escr, d_[0], decay_t)
                nc.vector.tensor_reduce(
                    out=partials[:, t:t + 1], in_=escr, op=OP.add,
                    axis=mybir.AxisListType.X,
                )

            nc.sync.dma_start(out, partials)

    nc.compile()
    _BUILD_CACHE[idx] = nc
    return nc


def kernel(feature, target, negative_features, idx):
    from concourse.bass_utils import run_bass_kernel_spmd

    feature = np.ascontiguousarray(np.asarray(feature, dtype=np.float32))
    target = np.asarray(target).astype(np.int64)
    negs = np.ascontiguousarray(np.asarray(negative_features, dtype=np.float32))
    idx_i = int(np.asarray(idx))

    negs16 = negs.astype(np.float16)
    onehot = (target[None, :] == np.arange(J)[:, None]).astype(np.float16)  # [J, N]
    maskL_full = (MASK_NEG * onehot).astype(np.float16)                     # [J, N]
    decay = (V ** np.arange(K, dtype=np.float64))
    decay = decay / decay.sum()
    decayb = np.broadcast_to(decay.astype(np.float32), (128, K)).copy()

    nc = _build(idx_i)
    in_maps = []
    for c in range(NCORES):
        sl = slice(c * NLOC, (c + 1) * NLOC)
        in_maps.append({
            "feat": np.ascontiguousarray(feature[sl]),
            "negs16": negs16,
            "maskL": np.ascontiguousarray(maskL_full[:, sl]),
            "onehotR": onehot,
            "decayb": decayb,
        })

    res = run_bass_kernel_spmd(nc, in_maps, core_ids=list(range(NCORES)))
    global LAST_RESULT
    LAST_RESULT = res
    total = 0.0
    for c in range(NCORES):
        total += float(np.asarray(res.results[c]["out"], dtype=np.float64).sum())
    loss = total / N + math.log(J)
    return np.float32(loss)


if __name__ == "__main__":
    rng = np.random.default_rng(0)
    f = rng.standard_normal((N, D)).astype(np.float32)
    ng = rng.standard_normal((J, N, D)).astype(np.float32)
    tg = rng.integers(0, J, size=N).astype(np.int64)
    print(kernel(f, tg, ng, 0))


# revision 19
# speedup vs baseline: 1.1026x; 1.1026x over previous
"""Trainium2 Bass kernel for nn_NegUniform (topk_masking).

Computes: L2-normalize feature & negative_features, sims = f_hat @ negs_hat^T
per negative set j (masked same-class for j==idx), top-16 per row, softmax
entropy over the J axis, decay-weighted mean + log(J).

Sharding: data-parallel over the n (row) dimension of `feature` across 8
NeuronCores; negative_features / target replicated. Each core returns
per-row-group partial sums [128, 4]; the host reduces them to the scalar.

Per-core pipeline:
  - negs (host-cast fp16) are DMA-xbar-transposed DRAM->SBUF to [d, m] layout
  - column norms via ACT square + PE ones-matmul, rsqrt on small tiles,
    partition-broadcast, one fp16 multiply -> normalized negsT
  - feature slice normalized in f32, cast fp16, xbar-transposed -> fT
  - sims tile [128 rows, 1024 cands] = fp16 matmul into PSUM f32; same-class
    mask folded in as a rank-4 one-hot matmul accumulated into PSUM (j==idx)
  - top-16 per row: DVE max8 per 1024-chunk directly from PSUM (union of
    top-8s), then max8 + match_replace + max8 over the 32 candidates
  - softmax-entropy over j in f32 on [128, 16] tiles, decay-weighted row sums
"""

import math
import sys

import numpy as np

for _p in ("/opt/trn_rl_repo",):
    if _p not in sys.path:
        sys.path.insert(0, _p)

N = 4096
D = 128
J = 4
NCORES = 8
NLOC = N // NCORES          # 512 rows per core
RT = NLOC // 128            # 4 row-tiles per core
K = 16
TEMP = 0.01
V = 0.95
MASK_NEG = -60000.0         # fp16-representable; dominates any cosine sim
CHUNK = 1024                # max8 scan chunk (2 PSUM banks)
NCHUNK = N // CHUNK         # 4 scan chunks per row-tile

_BUILD_CACHE = {}
LAST_RESULT = None  # BassKernelResults of the most recent kernel() call


def _build(idx: int):
    if idx in _BUILD_CACHE:
        return _BUILD_CACHE[idx]

    import concourse.bacc as bacc
    import concourse.tile as tile
    import concourse.mybir as mybir
    from concourse import bass_isa

    f32 = mybir.dt.float32
    f16 = mybir.dt.float16
    AF = mybir.ActivationFunctionType
    OP = mybir.AluOpType

    nc = bacc.Bacc(
        "TRN2",
        target_bir_lowering=False,
        debug=False,
        enable_asserts=False,
        num_devices=NCORES,
    )

    feat = nc.dram_tensor("feat", [NLOC, D], f32, kind="ExternalInput").ap()
    negs16 = nc.dram_tensor("negs16", [J, N, D], f16, kind="ExternalInput").ap()
    maskL = nc.dram_tensor("maskL", [J, NLOC], f16, kind="ExternalInput").ap()
    onehotR = nc.dram_tensor("onehotR", [J, N], f16, kind="ExternalInput").ap()
    decayb = nc.dram_tensor("decayb", [128, RT * K], f32, kind="ExternalInput").ap()
    out = nc.dram_tensor("out", [128, RT], f32, kind="ExternalOutput").ap()

    with tile.TileContext(nc) as tc:
        with (
            tc.tile_pool(name="consts", bufs=1) as cpool,
            tc.tile_pool(name="fprep", bufs=2) as fpool,
            tc.tile_pool(name="nprep", bufs=2) as npool,
            tc.tile_pool(name="negsT", bufs=2) as ntpool,
            tc.tile_pool(name="small", bufs=3) as spool,
            tc.tile_pool(name="tops", bufs=1) as tpool,
            tc.tile_pool(name="ent", bufs=1) as epool,
            tc.tile_pool(name="psums", bufs=3, space="PSUM") as psp,
            tc.tile_pool(name="psumn", bufs=2, space="PSUM") as pnp,
            tc.tile_pool(name="dramp", bufs=2, space="DRAM") as dpool,
        ):
            # ---- constants ----
            ones = cpool.tile([128, 1], f16)
            nc.vector.memset(ones, 1.0)
            decay_t = cpool.tile([128, RT * K], f32)
            nc.sync.dma_start(decay_t, decayb)
            maskL_t = cpool.tile([J, NLOC], f16)
            nc.sync.dma_start(maskL_t, maskL)
            onehotR_t = cpool.tile([J, N], f16)
            nc.sync.dma_start(onehotR_t, onehotR)
            partials = cpool.tile([128, RT], f32)

            # ---- feature prep: normalize f32, cast fp16, transpose ----
            fT = cpool.tile([128, NLOC], f16)  # [d, n_local]
            fhat_tiles = []
            fnrm2 = spool.tile([128, RT], f32, tag="fnrm")
            fscr = fpool.tile([128, D], f32, tag="fscr")
            ftiles = []
            for t in range(RT):
                ft = fpool.tile([128, D], f32, tag=f"ft{t}")
                nc.sync.dma_start(ft, feat[t * 128:(t + 1) * 128, :])
                ftiles.append(ft)
                nc.vector.tensor_mul(fscr, ft, ft)
                nc.vector.tensor_reduce(
                    out=fnrm2[:, t:t + 1], in_=fscr, op=OP.add,
                    axis=mybir.AxisListType.X,
                )
            fnrmS = spool.tile([128, RT], f32, tag="fnrmS")
            nc.scalar.activation(out=fnrmS, in_=fnrm2, func=AF.Sqrt)
            frs = spool.tile([128, RT], f32, tag="frs")
            nc.vector.reciprocal(frs, fnrmS)
            for t in range(RT):
                fh = fpool.tile([128, D], f16, tag=f"fh{t}")
                nc.vector.tensor_scalar(
                    out=fh, in0=ftiles[t], scalar1=frs[:, t:t + 1], scalar2=None,
                    op0=OP.mult,
                )
                fhat_tiles.append(fh)
                nc.sync.dma_start_transpose(fT[:, t * 128:(t + 1) * 128], fh)

            # ---- negs prep + main loop, per j ----
            topsJ = {}
            for j in range(J):
                raw = npool.tile([128, N], f16, tag="raw")  # [d, m] unnormalized
                for c in range(4):
                    nc.sync.dma_start_transpose(
                        raw[:, c * 1024:(c + 1) * 1024],
                        negs16[j, c * 1024:(c + 1) * 1024, :],
                    )
                sq = npool.tile([128, N], f16, tag="sq")
                nc.scalar.activation(out=sq, in_=raw, func=AF.Square)
                rsA = npool.tile([1, N], f32, tag="rsA")
                for c in range(8):
                    nps = pnp.tile([1, 512], f32, tag="nps")
                    nc.tensor.matmul(
                        nps, lhsT=ones, rhs=sq[:, c * 512:(c + 1) * 512],
                        start=True, stop=True,
                    )
                    nc.scalar.copy(rsA[:, c * 512:(c + 1) * 512], nps)
                nrm2 = npool.tile([32, 128], f32, tag="nrm2")
                nc.sync.dma_start(nrm2, rsA)  # [1,4096] -> [32,128]
                nrmS = npool.tile([32, 128], f32, tag="nrmS")
                nc.scalar.activation(out=nrmS, in_=nrm2, func=AF.Sqrt)
                rsf = npool.tile([32, 128], f32, tag="rsf")
                nc.vector.reciprocal(rsf, nrmS)
                rsh = npool.tile([32, 128], f16, tag="rsh")
                nc.vector.tensor_copy(rsh, rsf)
                rs1 = dpool.tile([1, N], f16, tag="rs1")
                nc.sync.dma_start(rs1, rsh)  # [32,128] -> [1,4096] fold (DRAM)
                rsb = npool.tile([128, N], f16, tag="rsb")
                nc.sync.dma_start(rsb, rs1.to_broadcast((128, N)))
                negsT = ntpool.tile([128, N], f16, tag="negsT")
                nc.vector.tensor_mul(negsT, raw, rsb)

                # ---- sims + topk for each row-tile ----
                top16 = tpool.tile([128, RT * K], f32, tag=f"topsJ{j}",
                                   name=f"topsJ{j}")
                topsJ[j] = top16
                for t in range(RT):
                    cand = spool.tile([128, 8 * NCHUNK], f32, tag="cand")
                    for c in range(NCHUNK):
                        ps = psp.tile([128, CHUNK], f32, tag="sims")
                        for h in range(CHUNK // 512):
                            m0 = c * CHUNK + h * 512
                            nc.tensor.matmul(
                                ps[:, h * 512:(h + 1) * 512],
                                lhsT=fT[:, t * 128:(t + 1) * 128],
                                rhs=negsT[:, m0:m0 + 512],
                                start=True, stop=(j != idx),
                            )
                            if j == idx:
                                nc.tensor.matmul(
                                    ps[:, h * 512:(h + 1) * 512],
                                    lhsT=maskL_t[:, t * 128:(t + 1) * 128],
                                    rhs=onehotR_t[:, m0:m0 + 512],
                                    start=False, stop=True,
                                )
                        nc.vector.max(out=cand[:, c * 8:(c + 1) * 8], in_=ps)
                    rep = spool.tile([128, 8 * NCHUNK], f32, tag="rep")
                    nc.vector.max(out=top16[:, t * K:t * K + 8], in_=cand)
                    nc.vector.match_replace(
                        out=rep, in_to_replace=top16[:, t * K:t * K + 8],
                        in_values=cand, imm_value=-1e30,
                    )
                    nc.vector.max(out=top16[:, t * K + 8:t * K + 16], in_=rep)

            # ---- softmax-entropy over j, decay-weighted row sums ----
            # All RT row-tiles at once on [128, RT*K] tiles.
            W = RT * K
            v = [topsJ[j] for j in range(J)]
            t01 = epool.tile([128, W], f32, tag="t01")
            t23 = epool.tile([128, W], f32, tag="t23")
            m = epool.tile([128, W], f32, tag="m")
            nc.vector.tensor_max(t01, v[0], v[1])
            nc.vector.tensor_max(t23, v[2], v[3])
            nc.vector.tensor_max(m, t01, t23)
            d_ = [epool.tile([128, W], f32, tag=f"d{j}", name=f"d{j}")
                  for j in range(J)]
            e_ = [epool.tile([128, W], f32, tag=f"e{j}", name=f"e{j}")
                  for j in range(J)]
            for j in range(J):
                nc.vector.tensor_sub(d_[j], v[j], m)
                nc.scalar.activation(out=e_[j], in_=d_[j], func=AF.Exp,
                                     scale=1.0 / TEMP)
            S = epool.tile([128, W], f32, tag="S")
            nc.vector.tensor_add(t01, e_[0], e_[1])
            nc.vector.tensor_add(t23, e_[2], e_[3])
            nc.vector.tensor_add(S, t01, t23)
            lnS = epool.tile([128, W], f32, tag="lnS")
            nc.scalar.activation(out=lnS, in_=S, func=AF.Ln)
            R = epool.tile([128, W], f32, tag="R")
            nc.vector.reciprocal(R, S)
            # w = sum_j e_j * d_j  (reuse d_ as scratch)
            for j in range(J):
                nc.vector.tensor_mul(d_[j], e_[j], d_[j])
            nc.vector.tensor_add(d_[0], d_[0], d_[1])
            nc.vector.tensor_add(d_[2], d_[2], d_[3])
            nc.vector.tensor_add(d_[0], d_[0], d_[2])
            # ent = (w * R) / TEMP - lnS
            nc.vector.tensor_mul(d_[0], d_[0], R)
            nc.vector.tensor_scalar(
                out=d_[0], in0=d_[0], scalar1=1.0 / TEMP, scalar2=None,
                op0=OP.mult,
            )
            nc.vector.tensor_sub(d_[0], d_[0], lnS)
            escr = epool.tile([128, W], f32, tag="escr")
            nc.vector.tensor_mul(escr, d_[0], decay_t)
            nc.vector.tensor_reduce(
                out=partials, in_=escr.rearrange("p (t k) -> p t k", k=K),
                op=OP.add, axis=mybir.AxisListType.X,
            )

            nc.sync.dma_start(out, partials)

    nc.compile()
    _BUILD_CACHE[idx] = nc
    return nc


def kernel(feature, target, negative_features, idx):
    from concourse.bass_utils import run_bass_kernel_spmd

    feature = np.ascontiguousarray(np.asarray(feature, dtype=np.float32))
    target = np.asarray(target).astype(np.int64)
    negs = np.ascontiguousarray(np.asarray(negative_features, dtype=np.float32))
    idx_i = int(np.asarray(idx))

    negs16 = negs.astype(np.float16)
    onehot = (target[None, :] == np.arange(J)[:, None]).astype(np.float16)  # [J, N]
    maskL_full = (MASK_NEG * onehot).astype(np.float16)                     # [J, N]
    decay = (V ** np.arange(K, dtype=np.float64))
    decay = decay / decay.sum()
    decay_row = np.tile(decay.astype(np.float32), RT)  # [RT*K]
    decayb = np.broadcast_to(decay_row, (128, RT * K)).copy()

    nc = _build(idx_i)
    in_maps = []
    for c in range(NCORES):
        sl = slice(c * NLOC, (c + 1) * NLOC)
        in_maps.append({
            "feat": np.ascontiguousarray(feature[sl]),
            "negs16": negs16,
            "maskL": np.ascontiguousarray(maskL_full[:, sl]),
            "onehotR": onehot,
            "decayb": decayb,
        })

    res = run_bass_kernel_spmd(nc, in_maps, core_ids=list(range(NCORES)))
    global LAST_RESULT
    LAST_RESULT = res
    total = 0.0
    for c in range(NCORES):
        total += float(np.asarray(res.results[c]["out"], dtype=np.float64).sum())
    loss = total / N + math.log(J)
    return np.float32(loss)


if __name__ == "__main__":
    rng = np.random.default_rng(0)
    f = rng.standard_normal((N, D)).astype(np.float32)
    ng = rng.standard_normal((J, N, D)).astype(np.float32)
    tg = rng.integers(0, J, size=N).astype(np.int64)
    print(kernel(f, tg, ng, 0))


# revision 21
# speedup vs baseline: 1.2653x; 1.1476x over previous
"""Trainium2 Bass kernel for nn_NegUniform (topk_masking).

Computes: L2-normalize feature & negative_features, sims = f_hat @ negs_hat^T
per negative set j (masked same-class for j==idx), top-16 per row, softmax
entropy over the J axis, decay-weighted mean + log(J).

Sharding: data-parallel over the n (row) dimension of `feature` across 8
NeuronCores; negative_features / target replicated. Each core returns
per-row-group partial sums [128, 4]; the host reduces them to the scalar.

Per-core pipeline:
  - negs (host-cast fp16) are DMA-xbar-transposed DRAM->SBUF to [d, m] layout
  - column norms via ACT square + PE ones-matmul, rsqrt on small tiles,
    partition-broadcast, one fp16 multiply -> normalized negsT
  - feature slice normalized in f32, cast fp16, xbar-transposed -> fT
  - sims tile [128 rows, 1024 cands] = fp16 matmul into PSUM f32; same-class
    mask folded in as a rank-4 one-hot matmul accumulated into PSUM (j==idx)
  - top-16 per row: DVE max8 per 1024-chunk directly from PSUM (union of
    top-8s), then max8 + match_replace + max8 over the 32 candidates
  - softmax-entropy over j in f32 on [128, 16] tiles, decay-weighted row sums
"""

import math
import sys

import numpy as np

for _p in ("/opt/trn_rl_repo",):
    if _p not in sys.path:
        sys.path.insert(0, _p)

N = 4096
D = 128
J = 4
NCORES = 8
NLOC = N // NCORES          # 512 rows per core
RT = NLOC // 128            # 4 row-tiles per core
K = 16
TEMP = 0.01
V = 0.95
MASK_NEG = -60000.0         # fp16-representable; dominates any cosine sim
CHUNK = 1024                # max8 scan chunk (2 PSUM banks)
NCHUNK = N // CHUNK         # 4 scan chunks per row-tile

_BUILD_CACHE = {}
LAST_RESULT = None  # BassKernelResults of the most recent kernel() call


def _build(idx: int):
    if idx in _BUILD_CACHE:
        return _BUILD_CACHE[idx]

    import concourse.bacc as bacc
    import concourse.tile as tile
    import concourse.mybir as mybir
    from concourse import bass_isa

    f32 = mybir.dt.float32
    f16 = mybir.dt.float16
    AF = mybir.ActivationFunctionType
    OP = mybir.AluOpType

    nc = bacc.Bacc(
        "TRN2",
        target_bir_lowering=False,
        debug=False,
        enable_asserts=False,
        num_devices=NCORES,
    )

    feat = nc.dram_tensor("feat", [NLOC, D], f32, kind="ExternalInput").ap()
    negs16 = nc.dram_tensor("negs16", [J, N, D], f16, kind="ExternalInput").ap()
    maskL = nc.dram_tensor("maskL", [J, NLOC], f16, kind="ExternalInput").ap()
    onehotR = nc.dram_tensor("onehotR", [J, N], f16, kind="ExternalInput").ap()
    decayb = nc.dram_tensor("decayb", [128, RT * K], f32, kind="ExternalInput").ap()
    out = nc.dram_tensor("out", [128, RT], f32, kind="ExternalOutput").ap()

    with tile.TileContext(nc) as tc:
        with (
            tc.tile_pool(name="consts", bufs=1) as cpool,
            tc.tile_pool(name="fprep", bufs=2) as fpool,
            tc.tile_pool(name="nprep", bufs=2) as npool,
            tc.tile_pool(name="negsT", bufs=2) as ntpool,
            tc.tile_pool(name="small", bufs=3) as spool,
            tc.tile_pool(name="tops", bufs=1) as tpool,
            tc.tile_pool(name="ent", bufs=1) as epool,
            tc.tile_pool(name="psums", bufs=3, space="PSUM") as psp,
            tc.tile_pool(name="psumn", bufs=2, space="PSUM") as pnp,
            tc.tile_pool(name="dramp", bufs=2, space="DRAM") as dpool,
        ):
            # ---- constants ----
            ones = cpool.tile([128, 1], f16)
            nc.vector.memset(ones, 1.0)
            decay_t = cpool.tile([128, RT * K], f32)
            nc.sync.dma_start(decay_t, decayb)
            maskL_t = cpool.tile([J, NLOC], f16)
            nc.sync.dma_start(maskL_t, maskL)
            onehotR_t = cpool.tile([J, N], f16)
            nc.sync.dma_start(onehotR_t, onehotR)
            partials = cpool.tile([128, RT], f32)

            # ---- feature prep: normalize f32, cast fp16, transpose ----
            fT = cpool.tile([128, NLOC], f16)  # [d, n_local]
            fhat_tiles = []
            fnrm2 = spool.tile([128, RT], f32, tag="fnrm")
            fscr = fpool.tile([128, D], f32, tag="fscr")
            ftiles = []
            for t in range(RT):
                ft = fpool.tile([128, D], f32, tag=f"ft{t}")
                nc.sync.dma_start(ft, feat[t * 128:(t + 1) * 128, :])
                ftiles.append(ft)
                nc.vector.tensor_mul(fscr, ft, ft)
                nc.vector.tensor_reduce(
                    out=fnrm2[:, t:t + 1], in_=fscr, op=OP.add,
                    axis=mybir.AxisListType.X,
                )
            fnrmS = spool.tile([128, RT], f32, tag="fnrmS")
            nc.scalar.activation(out=fnrmS, in_=fnrm2, func=AF.Sqrt)
            frs = spool.tile([128, RT], f32, tag="frs")
            nc.vector.reciprocal(frs, fnrmS)
            for t in range(RT):
                fh = fpool.tile([128, D], f16, tag=f"fh{t}")
                nc.vector.tensor_scalar(
                    out=fh, in0=ftiles[t], scalar1=frs[:, t:t + 1], scalar2=None,
                    op0=OP.mult,
                )
                fhat_tiles.append(fh)
                nc.sync.dma_start_transpose(fT[:, t * 128:(t + 1) * 128], fh)

            # ---- negs prep + main loop, per j ----
            topsJ = {}
            for j in range(J):
                raw = npool.tile([128, N], f16, tag="raw")  # [d, m] unnormalized
                for c in range(4):
                    nc.sync.dma_start_transpose(
                        raw[:, c * 1024:(c + 1) * 1024],
                        negs16[j, c * 1024:(c + 1) * 1024, :],
                    )
                sq = npool.tile([128, N], f16, tag="sq")
                nc.gpsimd.tensor_mul(sq, raw, raw)
                rsA = npool.tile([1, N], f32, tag="rsA")
                for c in range(8):
                    nps = pnp.tile([1, 512], f32, tag="nps")
                    nc.tensor.matmul(
                        nps, lhsT=ones, rhs=sq[:, c * 512:(c + 1) * 512],
                        start=True, stop=True,
                    )
                    nc.scalar.copy(rsA[:, c * 512:(c + 1) * 512], nps)
                nrm2 = npool.tile([128, 32], f32, tag="nrm2")
                nc.sync.dma_start(nrm2, rsA)  # [1,4096] -> [128,32]
                nrmS = npool.tile([128, 32], f32, tag="nrmS")
                nc.scalar.activation(out=nrmS, in_=nrm2, func=AF.Sqrt)
                rsf = npool.tile([128, 32], f32, tag="rsf")
                nc.vector.reciprocal(rsf, nrmS)
                rsh = npool.tile([128, 32], f16, tag="rsh")
                nc.vector.tensor_copy(rsh, rsf)
                rs1 = dpool.tile([1, N], f16, tag="rs1")
                nc.sync.dma_start(rs1, rsh)  # [128,32] -> [1,4096] fold (DRAM)
                rsb = npool.tile([128, N], f16, tag="rsb")
                nc.sync.dma_start(rsb, rs1.to_broadcast((128, N)))
                negsT = ntpool.tile([128, N], f16, tag="negsT")
                nc.gpsimd.tensor_mul(negsT, raw, rsb)

                # ---- sims + topk for each row-tile ----
                top16 = tpool.tile([128, RT * K], f32, tag=f"topsJ{j}",
                                   name=f"topsJ{j}")
                topsJ[j] = top16
                for t in range(RT):
                    cand = spool.tile([128, 8 * NCHUNK], f32, tag="cand")
                    for c in range(NCHUNK):
                        ps = psp.tile([128, CHUNK], f32, tag="sims")
                        for h in range(CHUNK // 512):
                            m0 = c * CHUNK + h * 512
                            nc.tensor.matmul(
                                ps[:, h * 512:(h + 1) * 512],
                                lhsT=fT[:, t * 128:(t + 1) * 128],
                                rhs=negsT[:, m0:m0 + 512],
                                start=True, stop=(j != idx),
                            )
                        if j == idx:
                            for h in range(CHUNK // 512):
                                m0 = c * CHUNK + h * 512
                                nc.tensor.matmul(
                                    ps[:, h * 512:(h + 1) * 512],
                                    lhsT=maskL_t[:, t * 128:(t + 1) * 128],
                                    rhs=onehotR_t[:, m0:m0 + 512],
                                    start=False, stop=True,
                                )
                        nc.vector.max(out=cand[:, c * 8:(c + 1) * 8], in_=ps)
                    rep = spool.tile([128, 8 * NCHUNK], f32, tag="rep")
                    nc.vector.max(out=top16[:, t * K:t * K + 8], in_=cand)
                    nc.vector.match_replace(
                        out=rep, in_to_replace=top16[:, t * K:t * K + 8],
                        in_values=cand, imm_value=-1e30,
                    )
                    nc.vector.max(out=top16[:, t * K + 8:t * K + 16], in_=rep)

            # ---- softmax-entropy over j, decay-weighted row sums ----
            # All RT row-tiles at once on [128, RT*K] tiles.
            W = RT * K
            v = [topsJ[j] for j in range(J)]
            t01 = epool.tile([128, W], f32, tag="t01")
            t23 = epool.tile([128, W], f32, tag="t23")
            m = epool.tile([128, W], f32, tag="m")
            nc.vector.tensor_max(t01, v[0], v[1])
            nc.vector.tensor_max(t23, v[2], v[3])
            nc.vector.tensor_max(m, t01, t23)
            d_ = [epool.tile([128, W], f32, tag=f"d{j}", name=f"d{j}")
                  for j in range(J)]
            e_ = [epool.tile([128, W], f32, tag=f"e{j}", name=f"e{j}")
                  for j in range(J)]
            for j in range(J):
                nc.vector.tensor_sub(d_[j], v[j], m)
                nc.scalar.activation(out=e_[j], in_=d_[j], func=AF.Exp,
                                     scale=1.0 / TEMP)
            S = epool.tile([128, W], f32, tag="S")
            nc.vector.tensor_add(t01, e_[0], e_[1])
            nc.vector.tensor_add(t23, e_[2], e_[3])
            nc.vector.tensor_add(S, t01, t23)
            lnS = epool.tile([128, W], f32, tag="lnS")
            nc.scalar.activation(out=lnS, in_=S, func=AF.Ln)
            R = epool.tile([128, W], f32, tag="R")
            nc.vector.reciprocal(R, S)
            # w = sum_j e_j * d_j  (reuse d_ as scratch)
            for j in range(J):
                nc.vector.tensor_mul(d_[j], e_[j], d_[j])
            nc.vector.tensor_add(d_[0], d_[0], d_[1])
            nc.vector.tensor_add(d_[2], d_[2], d_[3])
            nc.vector.tensor_add(d_[0], d_[0], d_[2])
            # ent = (w * R) / TEMP - lnS
            nc.vector.tensor_mul(d_[0], d_[0], R)
            nc.vector.tensor_scalar(
                out=d_[0], in0=d_[0], scalar1=1.0 / TEMP, scalar2=None,
                op0=OP.mult,
            )
            nc.vector.tensor_sub(d_[0], d_[0], lnS)
            escr = epool.tile([128, W], f32, tag="escr")
            nc.vector.tensor_mul(escr, d_[0], decay_t)
            nc.vector.tensor_reduce(
                out=partials, in_=escr.rearrange("p (t k) -> p t k", k=K),
                op=OP.add, axis=mybir.AxisListType.X,
            )

            nc.sync.dma_start(out, partials)

    nc.compile()
    _BUILD_CACHE[idx] = nc
    return nc


def kernel(feature, target, negative_features, idx):
    from concourse.bass_utils import run_bass_kernel_spmd

    feature = np.ascontiguousarray(np.asarray(feature, dtype=np.float32))
    target = np.asarray(target).astype(np.int64)
    negs = np.ascontiguousarray(np.asarray(negative_features, dtype=np.float32))
    idx_i = int(np.asarray(idx))

    negs16 = negs.astype(np.float16)
    onehot = (target[None, :] == np.arange(J)[:, None]).astype(np.float16)  # [J, N]
    maskL_full = (MASK_NEG * onehot).astype(np.float16)                     # [J, N]
    decay = (V ** np.arange(K, dtype=np.float64))
    decay = decay / decay.sum()
    decay_row = np.tile(decay.astype(np.float32), RT)  # [RT*K]
    decayb = np.broadcast_to(decay_row, (128, RT * K)).copy()

    nc = _build(idx_i)
    in_maps = []
    for c in range(NCORES):
        sl = slice(c * NLOC, (c + 1) * NLOC)
        in_maps.append({
            "feat": np.ascontiguousarray(feature[sl]),
            "negs16": negs16,
            "maskL": np.ascontiguousarray(maskL_full[:, sl]),
            "onehotR": onehot,
            "decayb": decayb,
        })

    res = run_bass_kernel_spmd(nc, in_maps, core_ids=list(range(NCORES)))
    global LAST_RESULT
    LAST_RESULT = res
    total = 0.0
    for c in range(NCORES):
        total += float(np.asarray(res.results[c]["out"], dtype=np.float64).sum())
    loss = total / N + math.log(J)
    return np.float32(loss)


if __name__ == "__main__":
    rng = np.random.default_rng(0)
    f = rng.standard_normal((N, D)).astype(np.float32)
    ng = rng.standard_normal((J, N, D)).astype(np.float32)
    tg = rng.integers(0, J, size=N).astype(np.int64)
    print(kernel(f, tg, ng, 0))


# revision 23
# speedup vs baseline: 1.3754x; 1.0870x over previous
"""Trainium2 Bass kernel for nn_NegUniform (topk_masking).

Computes: L2-normalize feature & negative_features, sims = f_hat @ negs_hat^T
per negative set j (masked same-class for j==idx), top-16 per row, softmax
entropy over the J axis, decay-weighted mean + log(J).

Sharding: data-parallel over the n (row) dimension of `feature` across 8
NeuronCores; negative_features / target replicated. Each core returns
per-row-group partial sums [128, 4]; the host reduces them to the scalar.

Per-core pipeline:
  - negs (host-cast fp16) are DMA-xbar-transposed DRAM->SBUF to [d, m] layout
  - column norms via ACT square + PE ones-matmul, rsqrt on small tiles,
    partition-broadcast, one fp16 multiply -> normalized negsT
  - feature slice normalized in f32, cast fp16, xbar-transposed -> fT
  - sims tile [128 rows, 1024 cands] = fp16 matmul into PSUM f32; same-class
    mask folded in as a rank-4 one-hot matmul accumulated into PSUM (j==idx)
  - top-16 per row: DVE max8 per 1024-chunk directly from PSUM (union of
    top-8s), then max8 + match_replace + max8 over the 32 candidates
  - softmax-entropy over j in f32 on [128, 16] tiles, decay-weighted row sums
"""

import math
import sys

import numpy as np

for _p in ("/opt/trn_rl_repo",):
    if _p not in sys.path:
        sys.path.insert(0, _p)

N = 4096
D = 128
J = 4
NCORES = 8
NLOC = N // NCORES          # 512 rows per core
RT = NLOC // 128            # 4 row-tiles per core
K = 16
TEMP = 0.01
V = 0.95
MASK_NEG = -60000.0         # fp16-representable; dominates any cosine sim
CHUNK = 1024                # max8 scan chunk (2 PSUM banks)
NCHUNK = N // CHUNK         # 4 scan chunks per row-tile

_BUILD_CACHE = {}
LAST_RESULT = None  # BassKernelResults of the most recent kernel() call


def _build(idx: int):
    if idx in _BUILD_CACHE:
        return _BUILD_CACHE[idx]

    import concourse.bacc as bacc
    import concourse.tile as tile
    import concourse.mybir as mybir
    from concourse import bass_isa

    f32 = mybir.dt.float32
    f16 = mybir.dt.float16
    AF = mybir.ActivationFunctionType
    OP = mybir.AluOpType

    nc = bacc.Bacc(
        "TRN2",
        target_bir_lowering=False,
        debug=False,
        enable_asserts=False,
        num_devices=NCORES,
    )

    feat = nc.dram_tensor("feat", [NLOC, D], f32, kind="ExternalInput").ap()
    negs16 = nc.dram_tensor("negs16", [J, N, D], f16, kind="ExternalInput").ap()
    maskL = nc.dram_tensor("maskL", [J, NLOC], f16, kind="ExternalInput").ap()
    onehotR = nc.dram_tensor("onehotR", [J, N], f16, kind="ExternalInput").ap()
    decayb = nc.dram_tensor("decayb", [128, RT * K], f32, kind="ExternalInput").ap()
    out = nc.dram_tensor("out", [128, RT], f32, kind="ExternalOutput").ap()

    with tile.TileContext(nc) as tc:
        with (
            tc.tile_pool(name="consts", bufs=1) as cpool,
            tc.tile_pool(name="fprep", bufs=2) as fpool,
            tc.tile_pool(name="nprep", bufs=2) as npool,
            tc.tile_pool(name="negsT", bufs=1) as ntpool,
            tc.tile_pool(name="small", bufs=3) as spool,
            tc.tile_pool(name="tops", bufs=1) as tpool,
            tc.tile_pool(name="ent", bufs=1) as epool,
            tc.tile_pool(name="psums", bufs=3, space="PSUM") as psp,
            tc.tile_pool(name="psumn", bufs=2, space="PSUM") as pnp,
            tc.tile_pool(name="dramp", bufs=2, space="DRAM") as dpool,
        ):
            # ---- constants ----
            ones = cpool.tile([128, 1], f16)
            nc.vector.memset(ones, 1.0)
            decay_t = cpool.tile([128, RT * K], f32)
            nc.sync.dma_start(decay_t, decayb)
            maskL_t = cpool.tile([J, NLOC], f16)
            nc.sync.dma_start(maskL_t, maskL)
            onehotR_t = cpool.tile([J, N], f16)
            nc.sync.dma_start(onehotR_t, onehotR)
            partials = cpool.tile([128, RT], f32)

            # ---- feature prep: normalize f32, cast fp16, transpose ----
            fT = cpool.tile([128, NLOC], f16)  # [d, n_local]
            fhat_tiles = []
            fnrm2 = spool.tile([128, RT], f32, tag="fnrm")
            fscr = fpool.tile([128, D], f32, tag="fscr")
            ftiles = []
            for t in range(RT):
                ft = fpool.tile([128, D], f32, tag=f"ft{t}")
                nc.sync.dma_start(ft, feat[t * 128:(t + 1) * 128, :])
                ftiles.append(ft)
                nc.vector.tensor_mul(fscr, ft, ft)
                nc.vector.tensor_reduce(
                    out=fnrm2[:, t:t + 1], in_=fscr, op=OP.add,
                    axis=mybir.AxisListType.X,
                )
            fnrmS = spool.tile([128, RT], f32, tag="fnrmS")
            nc.scalar.activation(out=fnrmS, in_=fnrm2, func=AF.Sqrt)
            frs = spool.tile([128, RT], f32, tag="frs")
            nc.vector.reciprocal(frs, fnrmS)
            for t in range(RT):
                fh = fpool.tile([128, D], f16, tag=f"fh{t}")
                nc.vector.tensor_scalar(
                    out=fh, in0=ftiles[t], scalar1=frs[:, t:t + 1], scalar2=None,
                    op0=OP.mult,
                )
                fhat_tiles.append(fh)
                nc.sync.dma_start_transpose(fT[:, t * 128:(t + 1) * 128], fh)

            # ---- negs prep, all j up front (pipelines with scan loop) ----
            topsJ = {}
            negsTs = {}
            for j in range(J):
                raw = npool.tile([128, N], f16, tag="raw", name=f"raw{j}")
                for c in range(4):
                    eng = nc.sync if (c % 2 == 0) else nc.scalar
                    eng.dma_start_transpose(
                        raw[:, c * 1024:(c + 1) * 1024],
                        negs16[j, c * 1024:(c + 1) * 1024, :],
                    )
                sq = npool.tile([128, N], f16, tag="sq", name=f"sq{j}")
                nc.scalar.activation(out=sq, in_=raw, func=AF.Square)
                rsA = npool.tile([1, N], f32, tag="rsA")
                for c in range(8):
                    nps = pnp.tile([1, 512], f32, tag="nps")
                    nc.tensor.matmul(
                        nps, lhsT=ones, rhs=sq[:, c * 512:(c + 1) * 512],
                        start=True, stop=True,
                    )
                    nc.scalar.copy(rsA[:, c * 512:(c + 1) * 512], nps)
                nrm2 = npool.tile([128, 32], f32, tag="nrm2")
                nc.sync.dma_start(nrm2, rsA)  # [1,4096] -> [128,32]
                nrmS = npool.tile([128, 32], f32, tag="nrmS")
                nc.scalar.activation(out=nrmS, in_=nrm2, func=AF.Sqrt)
                rsf = npool.tile([128, 32], f32, tag="rsf")
                nc.vector.reciprocal(rsf, nrmS)
                rsh = npool.tile([128, 32], f16, tag="rsh")
                nc.vector.tensor_copy(rsh, rsf)
                rs1 = dpool.tile([1, N], f16, tag="rs1")
                nc.sync.dma_start(rs1, rsh)  # [128,32] -> [1,4096] fold (DRAM)
                rsb = npool.tile([128, N], f16, tag="rsb", name=f"rsb{j}")
                nc.sync.dma_start(rsb, rs1.to_broadcast((128, N)))
                negsT = ntpool.tile([128, N], f16, tag=f"negsT{j}",
                                    name=f"negsT{j}")
                nc.vector.tensor_mul(negsT, raw, rsb)
                negsTs[j] = negsT

            # ---- sims + topk per (j, row-tile) ----
            for j in range(J):
                negsT = negsTs[j]
                top16 = tpool.tile([128, RT * K], f32, tag=f"topsJ{j}",
                                   name=f"topsJ{j}")
                topsJ[j] = top16
                for t in range(RT):
                    cand = spool.tile([128, 8 * NCHUNK], f32, tag="cand")
                    for c in range(NCHUNK):
                        ps = psp.tile([128, CHUNK], f32, tag="sims")
                        for h in range(CHUNK // 512):
                            m0 = c * CHUNK + h * 512
                            nc.tensor.matmul(
                                ps[:, h * 512:(h + 1) * 512],
                                lhsT=fT[:, t * 128:(t + 1) * 128],
                                rhs=negsT[:, m0:m0 + 512],
                                start=True, stop=(j != idx),
                            )
                        if j == idx:
                            for h in range(CHUNK // 512):
                                m0 = c * CHUNK + h * 512
                                nc.tensor.matmul(
                                    ps[:, h * 512:(h + 1) * 512],
                                    lhsT=maskL_t[:, t * 128:(t + 1) * 128],
                                    rhs=onehotR_t[:, m0:m0 + 512],
                                    start=False, stop=True,
                                )
                        nc.vector.max(out=cand[:, c * 8:(c + 1) * 8], in_=ps)
                    rep = spool.tile([128, 8 * NCHUNK], f32, tag="rep")
                    nc.vector.max(out=top16[:, t * K:t * K + 8], in_=cand)
                    nc.vector.match_replace(
                        out=rep, in_to_replace=top16[:, t * K:t * K + 8],
                        in_values=cand, imm_value=-1e30,
                    )
                    nc.vector.max(out=top16[:, t * K + 8:t * K + 16], in_=rep)

            # ---- softmax-entropy over j, decay-weighted row sums ----
            # All RT row-tiles at once on [128, RT*K] tiles.
            W = RT * K
            v = [topsJ[j] for j in range(J)]
            t01 = epool.tile([128, W], f32, tag="t01")
            t23 = epool.tile([128, W], f32, tag="t23")
            m = epool.tile([128, W], f32, tag="m")
            nc.vector.tensor_max(t01, v[0], v[1])
            nc.vector.tensor_max(t23, v[2], v[3])
            nc.vector.tensor_max(m, t01, t23)
            d_ = [epool.tile([128, W], f32, tag=f"d{j}", name=f"d{j}")
                  for j in range(J)]
            e_ = [epool.tile([128, W], f32, tag=f"e{j}", name=f"e{j}")
                  for j in range(J)]
            for j in range(J):
                nc.vector.tensor_sub(d_[j], v[j], m)
                nc.scalar.activation(out=e_[j], in_=d_[j], func=AF.Exp,
                                     scale=1.0 / TEMP)
            S = epool.tile([128, W], f32, tag="S")
            nc.vector.tensor_add(t01, e_[0], e_[1])
            nc.vector.tensor_add(t23, e_[2], e_[3])
            nc.vector.tensor_add(S, t01, t23)
            lnS = epool.tile([128, W], f32, tag="lnS")
            nc.scalar.activation(out=lnS, in_=S, func=AF.Ln)
            R = epool.tile([128, W], f32, tag="R")
            nc.vector.reciprocal(R, S)
            # w = sum_j e_j * d_j  (reuse d_ as scratch)
            for j in range(J):
                nc.vector.tensor_mul(d_[j], e_[j], d_[j])
            nc.vector.tensor_add(d_[0], d_[0], d_[1])
            nc.vector.tensor_add(d_[2], d_[2], d_[3])
            nc.vector.tensor_add(d_[0], d_[0], d_[2])
            # ent = (w * R) / TEMP - lnS
            nc.vector.tensor_mul(d_[0], d_[0], R)
            nc.vector.tensor_scalar(
                out=d_[0], in0=d_[0], scalar1=1.0 / TEMP, scalar2=None,
                op0=OP.mult,
            )
            nc.vector.tensor_sub(d_[0], d_[0], lnS)
            escr = epool.tile([128, W], f32, tag="escr")
            nc.vector.tensor_mul(escr, d_[0], decay_t)
            nc.vector.tensor_reduce(
                out=partials, in_=escr.rearrange("p (t k) -> p t k", k=K),
                op=OP.add, axis=mybir.AxisListType.X,
            )

            nc.sync.dma_start(out, partials)

    nc.compile()
    _BUILD_CACHE[idx] = nc
    return nc


def kernel(feature, target, negative_features, idx):
    from concourse.bass_utils import run_bass_kernel_spmd

    feature = np.ascontiguousarray(np.asarray(feature, dtype=np.float32))
    target = np.asarray(target).astype(np.int64)
    negs = np.ascontiguousarray(np.asarray(negative_features, dtype=np.float32))
    idx_i = int(np.asarray(idx))

    negs16 = negs.astype(np.float16)
    onehot = (target[None, :] == np.arange(J)[:, None]).astype(np.float16)  # [J, N]
    maskL_full = (MASK_NEG * onehot).astype(np.float16)                     # [J, N]
    decay = (V ** np.arange(K, dtype=np.float64))
    decay = decay / decay.sum()
    decay_row = np.tile(decay.astype(np.float32), RT)  # [RT*K]
    decayb = np.broadcast_to(decay_row, (128, RT * K)).copy()

    nc = _build(idx_i)
    in_maps = []
    for c in range(NCORES):
        sl = slice(c * NLOC, (c + 1) * NLOC)
        in_maps.append({
            "feat": np.ascontiguousarray(feature[sl]),
            "negs16": negs16,
            "maskL": np.ascontiguousarray(maskL_full[:, sl]),
            "onehotR": onehot,
            "decayb": decayb,
        })

    res = run_bass_kernel_spmd(nc, in_maps, core_ids=list(range(NCORES)))
    global LAST_RESULT
    LAST_RESULT = res
    total = 0.0
    for c in range(NCORES):
        total += float(np.asarray(res.results[c]["out"], dtype=np.float64).sum())
    loss = total / N + math.log(J)
    return np.float32(loss)


if __name__ == "__main__":
    rng = np.random.default_rng(0)
    f = rng.standard_normal((N, D)).astype(np.float32)
    ng = rng.standard_normal((J, N, D)).astype(np.float32)
    tg = rng.integers(0, J, size=N).astype(np.int64)
    print(kernel(f, tg, ng, 0))


# revision 24
# speedup vs baseline: 1.4827x; 1.0780x over previous
"""Trainium2 Bass kernel for nn_NegUniform (topk_masking).

Computes: L2-normalize feature & negative_features, sims = f_hat @ negs_hat^T
per negative set j (masked same-class for j==idx), top-16 per row, softmax
entropy over the J axis, decay-weighted mean + log(J).

Sharding: data-parallel over the n (row) dimension of `feature` across 8
NeuronCores; negative_features / target replicated. Each core returns
per-row-group partial sums [128, 4]; the host reduces them to the scalar.

Per-core pipeline:
  - negs (host-cast fp16) are DMA-xbar-transposed DRAM->SBUF to [d, m] layout
  - column norms via ACT square + PE ones-matmul, rsqrt on small tiles,
    partition-broadcast, one fp16 multiply -> normalized negsT
  - feature slice normalized in f32, cast fp16, xbar-transposed -> fT
  - sims tile [128 rows, 1024 cands] = fp16 matmul into PSUM f32; same-class
    mask folded in as a rank-4 one-hot matmul accumulated into PSUM (j==idx)
  - top-16 per row: DVE max8 per 1024-chunk directly from PSUM (union of
    top-8s), then max8 + match_replace + max8 over the 32 candidates
  - softmax-entropy over j in f32 on [128, 16] tiles, decay-weighted row sums
"""

import math
import sys

import numpy as np

for _p in ("/opt/trn_rl_repo",):
    if _p not in sys.path:
        sys.path.insert(0, _p)

N = 4096
D = 128
J = 4
NCORES = 8
NLOC = N // NCORES          # 512 rows per core
RT = NLOC // 128            # 4 row-tiles per core
K = 16
TEMP = 0.01
V = 0.95
MASK_NEG = -60000.0         # fp16-representable; dominates any cosine sim
CHUNK = 1024                # max8 scan chunk (2 PSUM banks)
NCHUNK = N // CHUNK         # 4 scan chunks per row-tile

_BUILD_CACHE = {}
LAST_RESULT = None  # BassKernelResults of the most recent kernel() call


def _build(idx: int):
    if idx in _BUILD_CACHE:
        return _BUILD_CACHE[idx]

    import concourse.bacc as bacc
    import concourse.tile as tile
    import concourse.mybir as mybir
    from concourse import bass_isa

    f32 = mybir.dt.float32
    f16 = mybir.dt.float16
    AF = mybir.ActivationFunctionType
    OP = mybir.AluOpType

    nc = bacc.Bacc(
        "TRN2",
        target_bir_lowering=False,
        debug=False,
        enable_asserts=False,
        num_devices=NCORES,
    )

    feat = nc.dram_tensor("feat", [NLOC, D], f32, kind="ExternalInput").ap()
    negs16 = nc.dram_tensor("negs16", [J, D, N], f16, kind="ExternalInput").ap()
    maskL = nc.dram_tensor("maskL", [J, NLOC], f16, kind="ExternalInput").ap()
    onehotR = nc.dram_tensor("onehotR", [J, N], f16, kind="ExternalInput").ap()
    decayb = nc.dram_tensor("decayb", [128, RT * K], f32, kind="ExternalInput").ap()
    out = nc.dram_tensor("out", [128, RT], f32, kind="ExternalOutput").ap()

    with tile.TileContext(nc) as tc:
        with (
            tc.tile_pool(name="consts", bufs=1) as cpool,
            tc.tile_pool(name="fprep", bufs=2) as fpool,
            tc.tile_pool(name="nprep", bufs=2) as npool,
            tc.tile_pool(name="negsT", bufs=1) as ntpool,
            tc.tile_pool(name="small", bufs=3) as spool,
            tc.tile_pool(name="tops", bufs=1) as tpool,
            tc.tile_pool(name="ent", bufs=1) as epool,
            tc.tile_pool(name="psums", bufs=3, space="PSUM") as psp,
            tc.tile_pool(name="psumn", bufs=2, space="PSUM") as pnp,
            tc.tile_pool(name="dramp", bufs=2, space="DRAM") as dpool,
        ):
            # ---- constants ----
            ones = cpool.tile([128, 1], f16)
            nc.vector.memset(ones, 1.0)
            decay_t = cpool.tile([128, RT * K], f32)
            nc.sync.dma_start(decay_t, decayb)
            maskL_t = cpool.tile([J, NLOC], f16)
            nc.sync.dma_start(maskL_t, maskL)
            onehotR_t = cpool.tile([J, N], f16)
            nc.sync.dma_start(onehotR_t, onehotR)
            partials = cpool.tile([128, RT], f32)

            # ---- feature prep: normalize f32, cast fp16, transpose ----
            fT = cpool.tile([128, NLOC], f16)  # [d, n_local]
            fhat_tiles = []
            fnrm2 = spool.tile([128, RT], f32, tag="fnrm")
            fscr = fpool.tile([128, D], f32, tag="fscr")
            ftiles = []
            for t in range(RT):
                ft = fpool.tile([128, D], f32, tag=f"ft{t}")
                nc.sync.dma_start(ft, feat[t * 128:(t + 1) * 128, :])
                ftiles.append(ft)
                nc.vector.tensor_mul(fscr, ft, ft)
                nc.vector.tensor_reduce(
                    out=fnrm2[:, t:t + 1], in_=fscr, op=OP.add,
                    axis=mybir.AxisListType.X,
                )
            fnrmS = spool.tile([128, RT], f32, tag="fnrmS")
            nc.scalar.activation(out=fnrmS, in_=fnrm2, func=AF.Sqrt)
            frs = spool.tile([128, RT], f32, tag="frs")
            nc.vector.reciprocal(frs, fnrmS)
            for t in range(RT):
                fh = fpool.tile([128, D], f16, tag=f"fh{t}")
                nc.vector.tensor_scalar(
                    out=fh, in0=ftiles[t], scalar1=frs[:, t:t + 1], scalar2=None,
                    op0=OP.mult,
                )
                fhat_tiles.append(fh)
                nc.sync.dma_start_transpose(fT[:, t * 128:(t + 1) * 128], fh)

            # ---- negs prep, all j up front (pipelines with scan loop) ----
            topsJ = {}
            negsTs = {}
            for j in range(J):
                raw = npool.tile([128, N], f16, tag="raw", name=f"raw{j}")
                for c in range(4):
                    eng = nc.sync if (c % 2 == 0) else nc.scalar
                    eng.dma_start(
                        raw[:, c * 1024:(c + 1) * 1024],
                        negs16[j, :, c * 1024:(c + 1) * 1024],
                    )
                sq = npool.tile([128, N], f16, tag="sq", name=f"sq{j}")
                if j == idx:
                    nc.scalar.activation(out=sq, in_=raw, func=AF.Square)
                else:
                    nc.gpsimd.tensor_mul(sq, raw, raw)
                rsA = npool.tile([1, N], f32, tag="rsA")
                for c in range(8):
                    nps = pnp.tile([1, 512], f32, tag="nps")
                    nc.tensor.matmul(
                        nps, lhsT=ones, rhs=sq[:, c * 512:(c + 1) * 512],
                        start=True, stop=True,
                    )
                    nc.scalar.copy(rsA[:, c * 512:(c + 1) * 512], nps)
                nrm2 = npool.tile([128, 32], f32, tag="nrm2")
                nc.sync.dma_start(nrm2, rsA)  # [1,4096] -> [128,32]
                nrmS = npool.tile([128, 32], f32, tag="nrmS")
                nc.scalar.activation(out=nrmS, in_=nrm2, func=AF.Sqrt)
                rsf = npool.tile([128, 32], f32, tag="rsf")
                nc.vector.reciprocal(rsf, nrmS)
                rsh = npool.tile([128, 32], f16, tag="rsh")
                nc.vector.tensor_copy(rsh, rsf)
                rs1 = dpool.tile([1, N], f16, tag="rs1")
                nc.sync.dma_start(rs1, rsh)  # [128,32] -> [1,4096] fold (DRAM)
                rsb = npool.tile([128, N], f16, tag="rsb", name=f"rsb{j}")
                nc.sync.dma_start(rsb, rs1.to_broadcast((128, N)))
                negsT = ntpool.tile([128, N], f16, tag=f"negsT{j}",
                                    name=f"negsT{j}")
                if j == idx:
                    nc.vector.tensor_mul(negsT, raw, rsb)
                else:
                    nc.gpsimd.tensor_mul(negsT, raw, rsb)
                negsTs[j] = negsT

            # ---- sims + topk per (j, row-tile) ----
            for j in range(J):
                negsT = negsTs[j]
                top16 = tpool.tile([128, RT * K], f32, tag=f"topsJ{j}",
                                   name=f"topsJ{j}")
                topsJ[j] = top16
                for t in range(RT):
                    cand = spool.tile([128, 8 * NCHUNK], f32, tag="cand")
                    for c in range(NCHUNK):
                        ps = psp.tile([128, CHUNK], f32, tag="sims")
                        for h in range(CHUNK // 512):
                            m0 = c * CHUNK + h * 512
                            nc.tensor.matmul(
                                ps[:, h * 512:(h + 1) * 512],
                                lhsT=fT[:, t * 128:(t + 1) * 128],
                                rhs=negsT[:, m0:m0 + 512],
                                start=True, stop=(j != idx),
                            )
                        if j == idx:
                            for h in range(CHUNK // 512):
                                m0 = c * CHUNK + h * 512
                                nc.tensor.matmul(
                                    ps[:, h * 512:(h + 1) * 512],
                                    lhsT=maskL_t[:, t * 128:(t + 1) * 128],
                                    rhs=onehotR_t[:, m0:m0 + 512],
                                    start=False, stop=True,
                                )
                        nc.vector.max(out=cand[:, c * 8:(c + 1) * 8], in_=ps)
                    rep = spool.tile([128, 8 * NCHUNK], f32, tag="rep")
                    nc.vector.max(out=top16[:, t * K:t * K + 8], in_=cand)
                    nc.vector.match_replace(
                        out=rep, in_to_replace=top16[:, t * K:t * K + 8],
                        in_values=cand, imm_value=-1e30,
                    )
                    nc.vector.max(out=top16[:, t * K + 8:t * K + 16], in_=rep)

            # ---- softmax-entropy over j, decay-weighted row sums ----
            # All RT row-tiles at once on [128, RT*K] tiles.
            W = RT * K
            v = [topsJ[j] for j in range(J)]
            t01 = epool.tile([128, W], f32, tag="t01")
            t23 = epool.tile([128, W], f32, tag="t23")
            m = epool.tile([128, W], f32, tag="m")
            nc.vector.tensor_max(t01, v[0], v[1])
            nc.vector.tensor_max(t23, v[2], v[3])
            nc.vector.tensor_max(m, t01, t23)
            d_ = [epool.tile([128, W], f32, tag=f"d{j}", name=f"d{j}")
                  for j in range(J)]
            e_ = [epool.tile([128, W], f32, tag=f"e{j}", name=f"e{j}")
                  for j in range(J)]
            for j in range(J):
                nc.vector.tensor_sub(d_[j], v[j], m)
                nc.scalar.activation(out=e_[j], in_=d_[j], func=AF.Exp,
                                     scale=1.0 / TEMP)
            S = epool.tile([128, W], f32, tag="S")
            nc.vector.tensor_add(t01, e_[0], e_[1])
            nc.vector.tensor_add(t23, e_[2], e_[3])
            nc.vector.tensor_add(S, t01, t23)
            lnS = epool.tile([128, W], f32, tag="lnS")
            nc.scalar.activation(out=lnS, in_=S, func=AF.Ln)
            R = epool.tile([128, W], f32, tag="R")
            nc.vector.reciprocal(R, S)
            # w = sum_j e_j * d_j  (reuse d_ as scratch)
            for j in range(J):
                nc.vector.tensor_mul(d_[j], e_[j], d_[j])
            nc.vector.tensor_add(d_[0], d_[0], d_[1])
            nc.vector.tensor_add(d_[2], d_[2], d_[3])
            nc.vector.tensor_add(d_[0], d_[0], d_[2])
            # ent = (w * R) / TEMP - lnS
            nc.vector.tensor_mul(d_[0], d_[0], R)
            nc.vector.tensor_scalar(
                out=d_[0], in0=d_[0], scalar1=1.0 / TEMP, scalar2=None,
                op0=OP.mult,
            )
            nc.vector.tensor_sub(d_[0], d_[0], lnS)
            escr = epool.tile([128, W], f32, tag="escr")
            nc.vector.tensor_mul(escr, d_[0], decay_t)
            nc.vector.tensor_reduce(
                out=partials, in_=escr.rearrange("p (t k) -> p t k", k=K),
                op=OP.add, axis=mybir.AxisListType.X,
            )

            nc.sync.dma_start(out, partials)

    nc.compile()
    _BUILD_CACHE[idx] = nc
    return nc


def kernel(feature, target, negative_features, idx):
    from concourse.bass_utils import run_bass_kernel_spmd

    feature = np.ascontiguousarray(np.asarray(feature, dtype=np.float32))
    target = np.asarray(target).astype(np.int64)
    negs = np.ascontiguousarray(np.asarray(negative_features, dtype=np.float32))
    idx_i = int(np.asarray(idx))

    negs16 = np.ascontiguousarray(negs.astype(np.float16).transpose(0, 2, 1))
    onehot = (target[None, :] == np.arange(J)[:, None]).astype(np.float16)  # [J, N]
    maskL_full = (MASK_NEG * onehot).astype(np.float16)                     # [J, N]
    decay = (V ** np.arange(K, dtype=np.float64))
    decay = decay / decay.sum()
    decay_row = np.tile(decay.astype(np.float32), RT)  # [RT*K]
    decayb = np.broadcast_to(decay_row, (128, RT * K)).copy()

    nc = _build(idx_i)
    in_maps = []
    for c in range(NCORES):
        sl = slice(c * NLOC, (c + 1) * NLOC)
        in_maps.append({
            "feat": np.ascontiguousarray(feature[sl]),
            "negs16": negs16,
            "maskL": np.ascontiguousarray(maskL_full[:, sl]),
            "onehotR": onehot,
            "decayb": decayb,
        })

    res = run_bass_kernel_spmd(nc, in_maps, core_ids=list(range(NCORES)))
    global LAST_RESULT
    LAST_RESULT = res
    total = 0.0
    for c in range(NCORES):
        total += float(np.asarray(res.results[c]["out"], dtype=np.float64).sum())
    loss = total / N + math.log(J)
    return np.float32(loss)


if __name__ == "__main__":
    rng = np.random.default_rng(0)
    f = rng.standard_normal((N, D)).astype(np.float32)
    ng = rng.standard_normal((J, N, D)).astype(np.float32)
    tg = rng.integers(0, J, size=N).astype(np.int64)
    print(kernel(f, tg, ng, 0))
